# Initial kernel scaffold
#
import numpy as np
import ml_dtypes

BF16 = ml_dtypes.bfloat16

DIM = 768
NB = 8
BS = 96
LAM = 0.01
B_FULL = 4
H = 128
W = 128
WF = W // 2 + 1
NBL = 4
C = NBL * BS
N_CORES = 8
HC = 4


def _host_consts():
    jh = np.arange(H)
    F = np.exp(-2j * np.pi * np.outer(jh, jh) / H)
    R = np.exp(-2j * np.pi * np.outer(np.arange(WF), np.arange(W)) / W) / 128.0
    RrT, RiT = R.real.T, R.imag.T
    FH = np.conj(F)
    cw = np.ones(WF)
    cw[1:-1] = 2.0
    S = (cw[:, None] * np.exp(2j * np.pi * np.outer(np.arange(WF), np.arange(W)) / W)) / 128.0
    consts = {
        "cF": np.concatenate([F.real, F.imag], 1).astype(np.float32),
        "cB1": np.concatenate([RrT, RiT], 1).astype(BF16),
        "cB2": np.concatenate([-RiT, RrT], 1).astype(BF16),
        "cE1": np.concatenate([S.real, S.imag], 1).astype(BF16),
        "cE2": np.concatenate([-S.imag, S.real], 1).astype(BF16),
        "cDr": FH.real.astype(BF16),
        "cDi": (-FH.imag).astype(BF16),
        "cI": np.eye(128, dtype=np.float32),
    }
    return consts


def _build_program():
    from contextlib import ExitStack

    import concourse.bass as bass
    import concourse.mybir as mybir
    import concourse.tile as tile
    from concourse import bacc

    f32 = mybir.dt.float32
    bf = mybir.dt.bfloat16
    AF = mybir.ActivationFunctionType

    nc = bacc.Bacc("TRN2", target_bir_lowering=False, debug=False)

    xs = nc.dram_tensor("xs", [C, H, W], f32, kind="ExternalInput")
    tb = nc.dram_tensor("tb", [DIM], f32, kind="ExternalInput")
    w1s = nc.dram_tensor("w1s", [2, NBL, BS, BS], f32, kind="ExternalInput")
    b1s = nc.dram_tensor("b1s", [2, NBL, BS], f32, kind="ExternalInput")
    w2s = nc.dram_tensor("w2s", [2, NBL, BS, BS], f32, kind="ExternalInput")
    b2s = nc.dram_tensor("b2s", [2, NBL, BS], f32, kind="ExternalInput")
    mwT = nc.dram_tensor("mwT", [DIM, 2 * NBL * BS], f32, kind="ExternalInput")
    mbs = nc.dram_tensor("mbs", [2 * NBL * BS], f32, kind="ExternalInput")
    cF = nc.dram_tensor("cF", [H, 2 * H], f32, kind="ExternalInput")
    cB1 = nc.dram_tensor("cB1", [W, 2 * WF], bf, kind="ExternalInput")
    cB2 = nc.dram_tensor("cB2", [W, 2 * WF], bf, kind="ExternalInput")
    cE1 = nc.dram_tensor("cE1", [WF, 2 * W], bf, kind="ExternalInput")
    cE2 = nc.dram_tensor("cE2", [WF, 2 * W], bf, kind="ExternalInput")
    cDr = nc.dram_tensor("cDr", [H, H], bf, kind="ExternalInput")
    cDi = nc.dram_tensor("cDi", [H, H], bf, kind="ExternalInput")
    cI = nc.dram_tensor("cI", [128, 128], f32, kind="ExternalInput")
    outs = nc.dram_tensor("outs", [C, H, W], f32, kind="ExternalOutput")

    with ExitStack() as ctx:
        tc = ctx.enter_context(tile.TileContext(nc))
        consts = ctx.enter_context(tc.tile_pool(name="consts", bufs=1))
        blockp = ctx.enter_context(tc.tile_pool(name="blockp", bufs=1))
        xstage = ctx.enter_context(tc.tile_pool(name="xstage", bufs=1))
        mixp = ctx.enter_context(tc.tile_pool(name="mixp", bufs=3))
        outp = ctx.enter_context(tc.tile_pool(name="outp", bufs=3))
        psum = ctx.enter_context(tc.tile_pool(name="psum", bufs=2, space="PSUM"))

        cF_sb = consts.tile([H, 2 * H], f32)
        nc.sync.dma_start(cF_sb, cF)
        cB1_sb = consts.tile([W, 2 * WF], bf)
        nc.sync.dma_start(cB1_sb, cB1)
        cB2_sb = consts.tile([W, 2 * WF], bf)
        nc.sync.dma_start(cB2_sb, cB2)
        cE1_sb = consts.tile([WF, 2 * W], bf)
        nc.sync.dma_start(cE1_sb, cE1)
        cE2_sb = consts.tile([WF, 2 * W], bf)
        nc.sync.dma_start(cE2_sb, cE2)
        cDr_sb = consts.tile([H, H], bf)
        nc.sync.dma_start(cDr_sb, cDr)
        cDi_sb = consts.tile([H, H], bf)
        nc.sync.dma_start(cDi_sb, cDi)
        cI_sb = consts.tile([128, 128], f32)
        nc.sync.dma_start(cI_sb, cI)

        w1r_sb = consts.tile([BS, NBL, BS], f32)
        w1i_sb = consts.tile([BS, NBL, BS], f32)
        nw1i_sb = consts.tile([BS, NBL, BS], f32)
        w2r_sb = consts.tile([BS, NBL, BS], f32)
        w2i_sb = consts.tile([BS, NBL, BS], f32)
        nw2i_sb = consts.tile([BS, NBL, BS], f32)
        nc.sync.dma_start(w1r_sb, w1s[0].rearrange("n d k -> d n k"))
        nc.sync.dma_start(w1i_sb, w1s[1].rearrange("n d k -> d n k"))
        nc.sync.dma_start(w2r_sb, w2s[0].rearrange("n d k -> d n k"))
        nc.sync.dma_start(w2i_sb, w2s[1].rearrange("n d k -> d n k"))
        nc.scalar.mul(nw1i_sb, w1i_sb, -1.0)
        nc.scalar.mul(nw2i_sb, w2i_sb, -1.0)

        t_sb = consts.tile([128, 6], f32)
        nc.sync.dma_start(t_sb, tb.rearrange("(j p) -> p j", p=128))
        s_sb = consts.tile([128, 6], f32)
        nc.scalar.activation(s_sb, t_sb, AF.Silu)
        mwT_sb = consts.tile([128, 6, 2 * NBL * BS], f32)
        nc.sync.dma_start(mwT_sb, mwT.rearrange("(uc p) j -> p uc j", p=128))
        mb_sb = consts.tile([1, 2 * NBL * BS], f32)
        nc.sync.dma_start(mb_sb, mbs.rearrange("j -> 1 j"))
        mod_sb = consts.tile([1, 2 * NBL * BS], f32)
        for half in range(2):
            pm = psum.tile([1, 384], f32, tag="ps_m")
            for uc in range(6):
                nc.tensor.matmul(
                    pm,
                    lhsT=s_sb[:, uc : uc + 1],
                    rhs=mwT_sb[:, uc, half * 384 : (half + 1) * 384],
                    start=(uc == 0),
                    stop=(uc == 5),
                )
            nc.vector.tensor_add(
                mod_sb[:, half * 384 : (half + 1) * 384],
                pm,
                mb_sb[:, half * 384 : (half + 1) * 384],
            )

        shp1 = consts.tile([BS, NBL], f32)
        scv = consts.tile([BS, NBL], f32)
        addr_v = consts.tile([BS, NBL], f32)
        addi_v = consts.tile([BS, NBL], f32)
        b1r_v = consts.tile([BS, NBL], f32)
        b1i_v = consts.tile([BS, NBL], f32)
        b2r_v = consts.tile([BS, NBL], f32)
        b2iml_v = consts.tile([BS, NBL], f32)
        nb2iml_v = consts.tile([BS, NBL], f32)
        nc.sync.dma_start(b1r_v, b1s[0].rearrange("n d -> d n"))
        nc.sync.dma_start(b1i_v, b1s[1].rearrange("n d -> d n"))
        nc.sync.dma_start(b2r_v, b2s[0].rearrange("n d -> d n"))
        b2i_tmp = consts.tile([BS, NBL], f32)
        nc.sync.dma_start(b2i_tmp, b2s[1].rearrange("n d -> d n"))
        nc.scalar.add(b2iml_v, b2i_tmp, -LAM)
        tmp_nb = consts.tile([BS, NBL], f32)
        nc.scalar.mul(tmp_nb, b2i_tmp, -1.0)
        nc.scalar.add(nb2iml_v, tmp_nb, -LAM)
        for n in range(NBL):
            nc.sync.dma_start(
                shp1[:, n : n + 1], mod_sb[0:1, n * 192 : n * 192 + 96]
            )
            nc.sync.dma_start(
                scv[:, n : n + 1], mod_sb[0:1, n * 192 + 96 : n * 192 + 192]
            )
        nc.scalar.add(shp1, shp1, 1.0)
        nc.vector.tensor_mul(addr_v, b1r_v, shp1)
        nc.vector.tensor_add(addr_v, addr_v, scv)
        nc.vector.tensor_mul(addi_v, b1i_v, shp1)
        nc.vector.tensor_add(addi_v, addi_v, scv)

        for n in range(NBL):
            c0 = n * BS

            X_blk = blockp.tile([H, BS, W], f32, tag="xblk")
            for cg in range(6):
                sl = slice(cg * 16, (cg + 1) * 16)
                nc.sync.dma_start(
                    X_blk[:, sl, :],
                    xs[c0 + cg * 16 : c0 + (cg + 1) * 16].rearrange("c h w -> h c w"),
                )

            Zbuf = blockp.tile([W, BS, 2 * H], bf, tag="zpbuf")
            for c in range(BS):
                pA = psum.tile([128, 2 * H], f32, tag="ps_a")
                nc.tensor.matmul(pA, lhsT=X_blk[:, c, :], rhs=cF_sb, start=True, stop=True)
                if c % 2 == 0:
                    nc.vector.tensor_copy(Zbuf[:, c, :], pA)
                else:
                    nc.scalar.copy(Zbuf[:, c, :], pA)

            Wr_pl = blockp.tile([WF, BS, H], bf, tag="wrpl")
            Wi_pl = blockp.tile([WF, BS, H], bf, tag="wipl")
            for ch_i in range(H // HC):
                h0 = ch_i * HC
                arch = mixp.tile([BS, HC, 2 * WF], f32, tag="arch")
                for j in range(HC):
                    pB = psum.tile([BS, 2 * WF], f32, tag="ps_b")
                    nc.tensor.matmul(
                        pB, lhsT=Zbuf[:, :, h0 + j], rhs=cB1_sb, start=True, stop=False
                    )
                    nc.tensor.matmul(
                        pB, lhsT=Zbuf[:, :, H + h0 + j], rhs=cB2_sb, start=False, stop=True
                    )
                    if j % 2 == 0:
                        nc.vector.tensor_copy(arch[:, j, :], pB)
                    else:
                        nc.scalar.copy(arch[:, j, :], pB)
                Ar = arch[:, :, 0:WF]
                Ai = arch[:, :, WF : 2 * WF]
                p1r = psum.tile([BS, HC, WF], f32, tag="ps_m")
                nc.tensor.matmul(p1r, lhsT=w1r_sb[:, n, :], rhs=Ar, start=True, stop=False)
                nc.tensor.matmul(p1r, lhsT=nw1i_sb[:, n, :], rhs=Ai, start=False, stop=True)
                p1i = psum.tile([BS, HC, WF], f32, tag="ps_m")
                nc.tensor.matmul(p1i, lhsT=w1i_sb[:, n, :], rhs=Ar, start=True, stop=False)
                nc.tensor.matmul(p1i, lhsT=w1r_sb[:, n, :], rhs=Ai, start=False, stop=True)
                r1 = mixp.tile([BS, HC, WF], f32, tag="r1")
                i1 = mixp.tile([BS, HC, WF], f32, tag="i1")
                nc.scalar.activation(
                    r1, p1r, AF.Relu, bias=addr_v[:, n : n + 1], scale=shp1[:, n : n + 1]
                )
                nc.scalar.activation(
                    i1, p1i, AF.Relu, bias=addi_v[:, n : n + 1], scale=shp1[:, n : n + 1]
                )
                p2r = psum.tile([BS, HC, WF], f32, tag="ps_m")
                nc.tensor.matmul(p2r, lhsT=w2r_sb[:, n, :], rhs=r1, start=True, stop=False)
                nc.tensor.matmul(p2r, lhsT=nw2i_sb[:, n, :], rhs=i1, start=False, stop=True)
                r2b = mixp.tile([BS, HC, WF], f32, tag="r2b")
                nc.scalar.activation(r2b, p2r, AF.Identity, bias=b2r_v[:, n : n + 1])
                p2i = psum.tile([BS, HC, WF], f32, tag="ps_m")
                nc.tensor.matmul(p2i, lhsT=w2i_sb[:, n, :], rhs=r2b, start=True, stop=False)
                nc.tensor.matmul(p2i, lhsT=w2r_sb[:, n, :], rhs=i1, start=False, stop=True)
                R2 = mixp.tile([BS, HC, WF], f32, tag="R2")
                I2 = mixp.tile([BS, HC, WF], f32, tag="I2")
                sa = mixp.tile([BS, HC, WF], f32, tag="sa")
                sb_ = mixp.tile([BS, HC, WF], f32, tag="sb")
                nc.scalar.activation(sa, r2b, AF.Relu, bias=-LAM)
                nc.scalar.activation(sb_, r2b, AF.Relu, bias=-LAM, scale=-1.0)
                nc.vector.tensor_sub(R2, sa, sb_)
                sc_ = mixp.tile([BS, HC, WF], f32, tag="sc")
                sd_ = mixp.tile([BS, HC, WF], f32, tag="sd")
                nc.scalar.activation(
                    sc_, p2i, AF.Relu, bias=b2iml_v[:, n : n + 1]
                )
                nc.scalar.activation(
                    sd_, p2i, AF.Relu, bias=nb2iml_v[:, n : n + 1], scale=-1.0
                )
                nc.vector.tensor_sub(I2, sc_, sd_)
                for j in range(HC):
                    pTr = psum.tile([WF, BS], f32, tag="ps_t")
                    nc.tensor.transpose(pTr, R2[:, j, :], cI_sb[0:BS, 0:BS])
                    pTi = psum.tile([WF, BS], f32, tag="ps_t")
                    nc.tensor.transpose(pTi, I2[:, j, :], cI_sb[0:BS, 0:BS])
                    if j % 2 == 0:
                        nc.vector.tensor_copy(Wr_pl[:, :, h0 + j], pTr)
                        nc.scalar.copy(Wi_pl[:, :, h0 + j], pTi)
                    else:
                        nc.scalar.copy(Wr_pl[:, :, h0 + j], pTr)
                        nc.vector.tensor_copy(Wi_pl[:, :, h0 + j], pTi)

            Pbuf = blockp.tile([H, BS, 2 * H], bf, tag="zpbuf")
            for c in range(BS):
                pE = psum.tile([128, 2 * H], f32, tag="ps_a")
                nc.tensor.matmul(pE, lhsT=Wr_pl[:, c, :], rhs=cE1_sb, start=True, stop=False)
                nc.tensor.matmul(pE, lhsT=Wi_pl[:, c, :], rhs=cE2_sb, start=False, stop=True)
                if c % 2 == 0:
                    nc.vector.tensor_copy(Pbuf[:, c, :], pE)
                else:
                    nc.scalar.copy(Pbuf[:, c, :], pE)

            for g in range(BS // 4):
                cg0 = g * 4
                pD = psum.tile([H, 4, W], f32, tag="ps_a")
                nc.tensor.matmul(
                    pD, lhsT=cDr_sb, rhs=Pbuf[:, cg0 : cg0 + 4, 0:H], start=True, stop=False
                )
                nc.tensor.matmul(
                    pD, lhsT=cDi_sb, rhs=Pbuf[:, cg0 : cg0 + 4, H : 2 * H], start=False, stop=True
                )
                ot = outp.tile([H, 4, W], f32, tag="ot")
                nc.vector.tensor_add(ot, pD, X_blk[:, cg0 : cg0 + 4, :])
                for j in range(4):
                    nc.sync.dma_start(outs[c0 + cg0 + j], ot[:, j, :])

    return nc


_CACHE = {}


def _get_program():
    if "nc" not in _CACHE:
        _CACHE["nc"] = _build_program()
    return _CACHE["nc"]


def kernel(**inputs):
    x = np.asarray(inputs["x"], dtype=np.float32)
    t = np.asarray(inputs["t"], dtype=np.float32)
    w1 = np.asarray(inputs["w1"], dtype=np.float32)
    b1 = np.asarray(inputs["b1"], dtype=np.float32)
    w2 = np.asarray(inputs["w2"], dtype=np.float32)
    b2 = np.asarray(inputs["b2"], dtype=np.float32)
    mod_w = np.asarray(inputs["mod_w"], dtype=np.float32)
    mod_b = np.asarray(inputs["mod_b"], dtype=np.float32)

    from concourse.bass_utils import run_bass_kernel_spmd

    nc = _get_program()
    consts = _host_consts()

    in_maps = []
    for core in range(N_CORES):
        b = core // 2
        n0 = (core % 2) * NBL
        cs = slice(n0 * BS, n0 * BS + C)
        rs = slice(n0 * 2 * BS, (n0 + NBL) * 2 * BS)
        im = {
            "xs": np.ascontiguousarray(x[b, cs]),
            "tb": np.ascontiguousarray(t[b]),
            "w1s": np.ascontiguousarray(w1[:, n0 : n0 + NBL]),
            "b1s": np.ascontiguousarray(b1[:, n0 : n0 + NBL]),
            "w2s": np.ascontiguousarray(w2[:, n0 : n0 + NBL]),
            "b2s": np.ascontiguousarray(b2[:, n0 : n0 + NBL]),
            "mwT": np.ascontiguousarray(mod_w[rs].T),
            "mbs": np.ascontiguousarray(mod_b[rs]),
        }
        im.update(consts)
        in_maps.append(im)

    res = run_bass_kernel_spmd(nc, in_maps, core_ids=list(range(N_CORES)))

    out = np.empty((B_FULL, DIM, H, W), dtype=np.float32)
    for core in range(N_CORES):
        b = core // 2
        n0 = (core % 2) * NBL
        cs = slice(n0 * BS, n0 * BS + C)
        out[b, cs] = res.results[core]["outs"]
    return out


# revision 9
# speedup vs baseline: 1.4635x; 1.4635x over previous
import numpy as np
import ml_dtypes

BF16 = ml_dtypes.bfloat16

DIM = 768
NB = 8
BS = 96
LAM = 0.01
B_FULL = 4
H = 128
W = 128
WF = W // 2 + 1
NBL = 4
C = NBL * BS
N_CORES = 8
HC = 4


def _host_consts():
    jh = np.arange(H)
    F = np.exp(-2j * np.pi * np.outer(jh, jh) / H)
    R = np.exp(-2j * np.pi * np.outer(np.arange(WF), np.arange(W)) / W) / 128.0
    RrT, RiT = R.real.T, R.imag.T
    FH = np.conj(F)
    cw = np.ones(WF)
    cw[1:-1] = 2.0
    S = (cw[:, None] * np.exp(2j * np.pi * np.outer(np.arange(WF), np.arange(W)) / W)) / 128.0
    consts = {
        "cF": np.concatenate([F.real, F.imag], 1).astype(np.float32),
        "cB1": np.concatenate([RrT, RiT], 1).astype(BF16),
        "cB2": np.concatenate([-RiT, RrT], 1).astype(BF16),
        "cE1": np.concatenate([S.real, S.imag], 1).astype(BF16),
        "cE2": np.concatenate([-S.imag, S.real], 1).astype(BF16),
        "cDr": FH.real.astype(BF16),
        "cDi": (-FH.imag).astype(BF16),
        "cI": np.eye(128, dtype=np.float32),
    }
    return consts


def _build_program():
    import os as _os
    _stages = set(_os.environ.get("K_STAGES", "MABXTED").upper())
    _reps = int(_os.environ.get("K_REPS", "1"))
    from contextlib import ExitStack

    import concourse.bass as bass
    import concourse.mybir as mybir
    import concourse.tile as tile
    from concourse import bacc

    f32 = mybir.dt.float32
    bf = mybir.dt.bfloat16
    AF = mybir.ActivationFunctionType

    nc = bacc.Bacc("TRN2", target_bir_lowering=False, debug=False)

    xs = nc.dram_tensor("xs", [C, H, W], f32, kind="ExternalInput")
    tb = nc.dram_tensor("tb", [DIM], f32, kind="ExternalInput")
    w1s = nc.dram_tensor("w1s", [2, NBL, BS, BS], f32, kind="ExternalInput")
    b1s = nc.dram_tensor("b1s", [2, NBL, BS], f32, kind="ExternalInput")
    w2s = nc.dram_tensor("w2s", [2, NBL, BS, BS], f32, kind="ExternalInput")
    b2s = nc.dram_tensor("b2s", [2, NBL, BS], f32, kind="ExternalInput")
    mwT = nc.dram_tensor("mwT", [DIM, 2 * NBL * BS], f32, kind="ExternalInput")
    mbs = nc.dram_tensor("mbs", [2 * NBL * BS], f32, kind="ExternalInput")
    cF = nc.dram_tensor("cF", [H, 2 * H], f32, kind="ExternalInput")
    cB1 = nc.dram_tensor("cB1", [W, 2 * WF], bf, kind="ExternalInput")
    cB2 = nc.dram_tensor("cB2", [W, 2 * WF], bf, kind="ExternalInput")
    cE1 = nc.dram_tensor("cE1", [WF, 2 * W], bf, kind="ExternalInput")
    cE2 = nc.dram_tensor("cE2", [WF, 2 * W], bf, kind="ExternalInput")
    cDr = nc.dram_tensor("cDr", [H, H], bf, kind="ExternalInput")
    cDi = nc.dram_tensor("cDi", [H, H], bf, kind="ExternalInput")
    cI = nc.dram_tensor("cI", [128, 128], f32, kind="ExternalInput")
    outs = nc.dram_tensor("outs", [C, H, W], f32, kind="ExternalOutput")

    with ExitStack() as ctx:
        tc = ctx.enter_context(tile.TileContext(nc))
        consts = ctx.enter_context(tc.tile_pool(name="consts", bufs=1))
        blockp = ctx.enter_context(tc.tile_pool(name="blockp", bufs=1))
        xstage = ctx.enter_context(tc.tile_pool(name="xstage", bufs=1))
        mixp = ctx.enter_context(tc.tile_pool(name="mixp", bufs=2))
        outp = ctx.enter_context(tc.tile_pool(name="outp", bufs=3))
        psum = ctx.enter_context(tc.tile_pool(name="psum", bufs=2, space="PSUM"))

        cF_sb = consts.tile([H, 2 * H], f32)
        nc.sync.dma_start(cF_sb, cF[:])
        cB1_sb = consts.tile([W, 2 * WF], bf)
        nc.sync.dma_start(cB1_sb, cB1[:])
        cB2_sb = consts.tile([W, 2 * WF], bf)
        nc.sync.dma_start(cB2_sb, cB2[:])
        cE1_sb = consts.tile([WF, 2 * W], bf)
        nc.sync.dma_start(cE1_sb, cE1[:])
        cE2_sb = consts.tile([WF, 2 * W], bf)
        nc.sync.dma_start(cE2_sb, cE2[:])
        cDr_sb = consts.tile([H, H], bf)
        nc.sync.dma_start(cDr_sb, cDr[:])
        cDi_sb = consts.tile([H, H], bf)
        nc.sync.dma_start(cDi_sb, cDi[:])
        cI_sb = consts.tile([128, 128], f32)
        nc.sync.dma_start(cI_sb, cI[:])

        w1r_sb = consts.tile([BS, NBL, BS], f32)
        w1i_sb = consts.tile([BS, NBL, BS], f32)
        nw1i_sb = consts.tile([BS, NBL, BS], f32)
        w2r_sb = consts.tile([BS, NBL, BS], f32)
        w2i_sb = consts.tile([BS, NBL, BS], f32)
        nw2i_sb = consts.tile([BS, NBL, BS], f32)
        nc.sync.dma_start(w1r_sb, w1s[0].rearrange("n d k -> d n k"))
        nc.sync.dma_start(w1i_sb, w1s[1].rearrange("n d k -> d n k"))
        nc.sync.dma_start(w2r_sb, w2s[0].rearrange("n d k -> d n k"))
        nc.sync.dma_start(w2i_sb, w2s[1].rearrange("n d k -> d n k"))
        nc.scalar.mul(nw1i_sb, w1i_sb, -1.0)
        nc.scalar.mul(nw2i_sb, w2i_sb, -1.0)

        modpool_cm = tc.tile_pool(name="modp", bufs=1)
        modpool = modpool_cm.__enter__()
        t_sb = modpool.tile([128, 6], f32)
        nc.sync.dma_start(t_sb, tb[:].rearrange("(j p) -> p j", p=128))
        s_sb = modpool.tile([128, 6], f32)
        nc.scalar.activation(s_sb, t_sb, AF.Silu)
        mwT_sb = modpool.tile([128, 6, 2 * NBL * BS], f32)
        nc.sync.dma_start(mwT_sb, mwT[:].rearrange("(uc p) j -> p uc j", p=128))
        mb_sb = modpool.tile([1, 2 * NBL * BS], f32)
        nc.sync.dma_start(mb_sb, mbs[None, :])
        mod_sb = modpool.tile([1, 2 * NBL * BS], f32)
        nc.vector.memset(mod_sb, 0.0)
        for half in range(2 if "M" in _stages else 0):
            pm = psum.tile([1, 384], f32, tag="ps_m")
            for uc in range(6):
                nc.tensor.matmul(
                    pm,
                    lhsT=s_sb[:, uc : uc + 1],
                    rhs=mwT_sb[:, uc, half * 384 : (half + 1) * 384],
                    start=(uc == 0),
                    stop=(uc == 5),
                )
            nc.vector.tensor_add(
                mod_sb[:, half * 384 : (half + 1) * 384],
                pm,
                mb_sb[:, half * 384 : (half + 1) * 384],
            )

        lamn = consts.tile([128, 1], f32)
        nc.vector.memset(lamn, -LAM)
        shp1 = consts.tile([BS, NBL], f32)
        scv = consts.tile([BS, NBL], f32)
        addr_v = consts.tile([BS, NBL], f32)
        addi_v = consts.tile([BS, NBL], f32)
        b1r_v = consts.tile([BS, NBL], f32)
        b1i_v = consts.tile([BS, NBL], f32)
        b2r_v = consts.tile([BS, NBL], f32)
        b2iml_v = consts.tile([BS, NBL], f32)
        nb2iml_v = consts.tile([BS, NBL], f32)
        nc.sync.dma_start(b1r_v, b1s[0].rearrange("n d -> d n"))
        nc.sync.dma_start(b1i_v, b1s[1].rearrange("n d -> d n"))
        nc.sync.dma_start(b2r_v, b2s[0].rearrange("n d -> d n"))
        b2i_tmp = consts.tile([BS, NBL], f32)
        nc.sync.dma_start(b2i_tmp, b2s[1].rearrange("n d -> d n"))
        nc.scalar.add(b2iml_v, b2i_tmp, lamn[0:BS])
        tmp_nb = consts.tile([BS, NBL], f32)
        nc.scalar.mul(tmp_nb, b2i_tmp, -1.0)
        nc.scalar.add(nb2iml_v, tmp_nb, lamn[0:BS])
        for n in range(NBL):
            nc.sync.dma_start(
                shp1[:, n : n + 1], mod_sb[0:1, n * 192 : n * 192 + 96]
            )
            nc.sync.dma_start(
                scv[:, n : n + 1], mod_sb[0:1, n * 192 + 96 : n * 192 + 192]
            )
        nc.scalar.add(shp1, shp1, 1.0)
        nc.vector.tensor_mul(addr_v, b1r_v, shp1)
        nc.vector.tensor_add(addr_v, addr_v, scv)
        nc.vector.tensor_mul(addi_v, b1i_v, shp1)
        nc.vector.tensor_add(addi_v, addi_v, scv)
        modpool_cm.__exit__(None, None, None)

        rep_cm = tc.For_i(0, _reps, 1) if _reps > 1 else None
        if rep_cm is not None:
            rep_cm.__enter__()
        for n in range(NBL):
            c0 = n * BS

            X_blk = blockp.tile([H, BS, W], f32, tag="xblk")
            for cg in range(6):
                sl = slice(cg * 16, (cg + 1) * 16)
                nc.sync.dma_start(
                    X_blk[:, sl, :],
                    xs[c0 + cg * 16 : c0 + (cg + 1) * 16].rearrange("c h w -> h c w"),
                )

            Zbuf = blockp.tile([W, BS, 2 * H], bf, tag="zpbuf")
            for c in range(BS if "A" in _stages else 0):
                pA = psum.tile([128, 2 * H], f32, tag="ps_a")
                nc.tensor.matmul(pA, lhsT=X_blk[:, c, :], rhs=cF_sb, start=True, stop=True)
                if c % 2 == 0:
                    nc.vector.tensor_copy(Zbuf[:, c, :], pA)
                else:
                    nc.scalar.copy(Zbuf[:, c, :], pA)

            Wr_pl = blockp.tile([WF, BS, H], bf, tag="wrpl")
            Wi_pl = blockp.tile([WF, BS, H], bf, tag="wipl")
            for ch_i in range(H // HC if "B" in _stages else 0):
                h0 = ch_i * HC
                arch = mixp.tile([BS, HC, 2 * WF], f32, tag="arch")
                for j in range(HC):
                    pB = psum.tile([BS, 2 * WF], f32, tag="ps_b")
                    nc.tensor.matmul(
                        pB, lhsT=Zbuf[:, :, h0 + j], rhs=cB1_sb, start=True, stop=False
                    )
                    nc.tensor.matmul(
                        pB, lhsT=Zbuf[:, :, H + h0 + j], rhs=cB2_sb, start=False, stop=True
                    )
                    if j % 2 == 0:
                        nc.vector.tensor_copy(arch[:, j, :], pB)
                    else:
                        nc.scalar.copy(arch[:, j, :], pB)
                Ar = arch[:, :, 0:WF]
                Ai = arch[:, :, WF : 2 * WF]
                p1r = psum.tile([BS, HC, WF], f32, tag="ps_m")
                nc.tensor.matmul(p1r, lhsT=w1r_sb[:, n, :], rhs=Ar, start=True, stop=False)
                nc.tensor.matmul(p1r, lhsT=nw1i_sb[:, n, :], rhs=Ai, start=False, stop=True)
                p1i = psum.tile([BS, HC, WF], f32, tag="ps_m")
                nc.tensor.matmul(p1i, lhsT=w1i_sb[:, n, :], rhs=Ar, start=True, stop=False)
                nc.tensor.matmul(p1i, lhsT=w1r_sb[:, n, :], rhs=Ai, start=False, stop=True)
                r1 = mixp.tile([BS, HC, WF], f32, tag="r1")
                i1 = mixp.tile([BS, HC, WF], f32, tag="i1")
                nc.scalar.activation(
                    r1, p1r, AF.Relu, bias=addr_v[:, n : n + 1], scale=shp1[:, n : n + 1]
                )
                nc.scalar.activation(
                    i1, p1i, AF.Relu, bias=addi_v[:, n : n + 1], scale=shp1[:, n : n + 1]
                )
                p2r = psum.tile([BS, HC, WF], f32, tag="ps_m")
                nc.tensor.matmul(p2r, lhsT=w2r_sb[:, n, :], rhs=r1, start=True, stop=False)
                nc.tensor.matmul(p2r, lhsT=nw2i_sb[:, n, :], rhs=i1, start=False, stop=True)
                r2b = mixp.tile([BS, HC, WF], f32, tag="r2b")
                nc.scalar.activation(r2b, p2r, AF.Identity, bias=b2r_v[:, n : n + 1])
                p2i = psum.tile([BS, HC, WF], f32, tag="ps_m")
                nc.tensor.matmul(p2i, lhsT=w2i_sb[:, n, :], rhs=r2b, start=True, stop=False)
                nc.tensor.matmul(p2i, lhsT=w2r_sb[:, n, :], rhs=i1, start=False, stop=True)
                R2 = mixp.tile([BS, HC, WF], f32, tag="R2")
                I2 = mixp.tile([BS, HC, WF], f32, tag="I2")
                sa = mixp.tile([BS, HC, WF], f32, tag="shr_a")
                sb_ = mixp.tile([BS, HC, WF], f32, tag="shr_b")
                nc.scalar.activation(sa, r2b, AF.Relu, bias=lamn[0:BS])
                nc.scalar.activation(sb_, r2b, AF.Relu, bias=lamn[0:BS], scale=-1.0)
                nc.vector.tensor_sub(R2, sa, sb_)
                sc_ = mixp.tile([BS, HC, WF], f32, tag="shr_a")
                sd_ = mixp.tile([BS, HC, WF], f32, tag="shr_b")
                nc.scalar.activation(
                    sc_, p2i, AF.Relu, bias=b2iml_v[:, n : n + 1]
                )
                nc.scalar.activation(
                    sd_, p2i, AF.Relu, bias=nb2iml_v[:, n : n + 1], scale=-1.0
                )
                nc.vector.tensor_sub(I2, sc_, sd_)
                for j in range(HC):
                    pTr = psum.tile([WF, BS], f32, tag="ps_t")
                    nc.tensor.transpose(pTr, R2[:, j, :], cI_sb[0:BS, 0:BS])
                    pTi = psum.tile([WF, BS], f32, tag="ps_t")
                    nc.tensor.transpose(pTi, I2[:, j, :], cI_sb[0:BS, 0:BS])
                    if j % 2 == 0:
                        nc.vector.tensor_copy(Wr_pl[:, :, h0 + j], pTr)
                        nc.scalar.copy(Wi_pl[:, :, h0 + j], pTi)
                    else:
                        nc.scalar.copy(Wr_pl[:, :, h0 + j], pTr)
                        nc.vector.tensor_copy(Wi_pl[:, :, h0 + j], pTi)

            Pbuf = blockp.tile([H, BS, 2 * H], bf, tag="zpbuf")
            for c in range(BS if "E" in _stages else 0):
                pE = psum.tile([128, 2 * H], f32, tag="ps_a")
                nc.tensor.matmul(pE, lhsT=Wr_pl[:, c, :], rhs=cE1_sb, start=True, stop=False)
                nc.tensor.matmul(pE, lhsT=Wi_pl[:, c, :], rhs=cE2_sb, start=False, stop=True)
                if c % 2 == 0:
                    nc.vector.tensor_copy(Pbuf[:, c, :], pE)
                else:
                    nc.scalar.copy(Pbuf[:, c, :], pE)

            for g in range(BS // 4 if "D" in _stages else 0):
                cg0 = g * 4
                pD = psum.tile([H, 4, W], f32, tag="ps_a")
                nc.tensor.matmul(
                    pD, lhsT=cDr_sb, rhs=Pbuf[:, cg0 : cg0 + 4, 0:H], start=True, stop=False
                )
                nc.tensor.matmul(
                    pD, lhsT=cDi_sb, rhs=Pbuf[:, cg0 : cg0 + 4, H : 2 * H], start=False, stop=True
                )
                ot = outp.tile([H, 4, W], f32, tag="ot")
                nc.vector.tensor_add(ot, pD, X_blk[:, cg0 : cg0 + 4, :])
                for j in range(4):
                    nc.sync.dma_start(outs[c0 + cg0 + j], ot[:, j, :])

        if rep_cm is not None:
            rep_cm.__exit__(None, None, None)

    nc.compile()
    return nc


_CACHE = {}


def _get_program():
    if "nc" not in _CACHE:
        _CACHE["nc"] = _build_program()
    return _CACHE["nc"]


def kernel(**inputs):
    x = np.asarray(inputs["x"], dtype=np.float32)
    t = np.asarray(inputs["t"], dtype=np.float32)
    w1 = np.asarray(inputs["w1"], dtype=np.float32)
    b1 = np.asarray(inputs["b1"], dtype=np.float32)
    w2 = np.asarray(inputs["w2"], dtype=np.float32)
    b2 = np.asarray(inputs["b2"], dtype=np.float32)
    mod_w = np.asarray(inputs["mod_w"], dtype=np.float32)
    mod_b = np.asarray(inputs["mod_b"], dtype=np.float32)

    from concourse.bass_utils import run_bass_kernel_spmd

    nc = _get_program()
    consts = _host_consts()

    in_maps = []
    for core in range(N_CORES):
        b = core // 2
        n0 = (core % 2) * NBL
        cs = slice(n0 * BS, n0 * BS + C)
        rs = slice(n0 * 2 * BS, (n0 + NBL) * 2 * BS)
        im = {
            "xs": np.ascontiguousarray(x[b, cs]),
            "tb": np.ascontiguousarray(t[b]),
            "w1s": np.ascontiguousarray(w1[:, n0 : n0 + NBL]),
            "b1s": np.ascontiguousarray(b1[:, n0 : n0 + NBL]),
            "w2s": np.ascontiguousarray(w2[:, n0 : n0 + NBL]),
            "b2s": np.ascontiguousarray(b2[:, n0 : n0 + NBL]),
            "mwT": np.ascontiguousarray(mod_w[rs].T),
            "mbs": np.ascontiguousarray(mod_b[rs]),
        }
        im.update(consts)
        in_maps.append(im)

    import os as _os
    trace = bool(int(_os.environ.get("AFNO_TRACE", "0")))
    res = run_bass_kernel_spmd(
        nc, in_maps, core_ids=list(range(N_CORES)), trace=trace
    )
    if trace:
        print("exec_time_ns:", res.exec_time_ns,
              "mean:", res.mean_exec_time_ns,
              "max_core:", res.max_exec_time_core_id)
        if res.instructions_and_trace:
            print("trace path:", res.instructions_and_trace[1])
        _CACHE["last_profile"] = res

    out = np.empty((B_FULL, DIM, H, W), dtype=np.float32)
    for core in range(N_CORES):
        b = core // 2
        n0 = (core % 2) * NBL
        cs = slice(n0 * BS, n0 * BS + C)
        out[b, cs] = res.results[core]["outs"]
    return out
